# revision 32
# baseline (speedup 1.0000x reference)
"""HBV hydrological model (nn_HBVMulTDET_WaterLoss) as a Bass/Tile kernel on
8 Trainium2 NeuronCores.

Strategy: pure data parallelism over the 4000 grid cells (500 cells/core).
Per-core layout: partition p in [0,125) holds 4 cells x 4 components = 16
state lanes in the free dim. All state-free derived quantities (rain/snow
partitioning, melt/refreeze energy, scaled parameters, the log-space
constants of both soil pow() terms) are precomputed on the host and DMAd
directly, so the device program is a pure steady-state recurrence stream:
the T=365 step loop fully unrolled and balanced across the DVE, Pool
(GPSIMD) and Scalar (ACT) engines, with the ACT queue carrying ONLY the
four critical-path activations (Ln/Exp of the two soil pow chains).

Algebraic restructuring vs the reference (verified < 1e-4 abs):
  - snow melt/refreeze collapsed into one signed flux
        X = max(min(E, SP+SNOW), -MW),  E = melt_cap - refreeze_cap
    (exact: the two capacities are mutually exclusive by sign of Ta-TT);
    NZ floors on SP/MW dropped (bounded 1e-5 perturbation, verified);
    meltwater is carried negated (NMW) so the flux clamp is a plain max
  - soil pow() via exp/ln: (SM/FC)^BETA = exp(BETA*ln(SM) - BETA*ln(FC)),
    second pow fed by pre-excess SM1 (identical after the PET clip), with
    PET folded in: PET*evap = exp(BETAET*ln(SM1) + lnPET - BETAET*ln(LP*FC))
  - ET/SM update collapsed via SM3 = max(SMc - x2, max(SMc - PET, NZ))
  - capillary min() dropped (C <= 1 makes it redundant)
  - response: rech+exc == SMa-SMc, (1-K) folding with negated states
    (NSUZ = -SUZ, NSLZ = -SLZ), and Q0+Q1+Q2 == SUZ2+SLZ2+NSUZ'+NSLZ'
    accumulated in one strided-view tensor_reduce over a combined tile
All activations are forced into the single natural_log_exp_and_others
table set so the scalar engine never reloads its activation tables.
Gamma unit-hydrograph weights are computed on host; the routing
convolution runs on device.
"""
import math
import numpy as np

T_FULL = 365
NGRID = 4000
NCORES = 8
NSH = NGRID // NCORES      # 500 cells per core
PPART = 125                # partitions used
CL = 4                     # cells per partition
M = 4                      # nmul components
LENF = 15
NZ = 1e-5
TC = 32                    # time-chunk length

# host-precomputed per-step tensors, DMAd as dd[j]: [PPART, T, CL, M]
DD = ["E", "SNOW", "RAIN", "CWHn", "BETA", "BLF", "FC", "FCinv", "BETAET",
      "LNPB", "C", "PERC", "NUZL", "K0", "K1Cn", "K2Cn"]
DJ = {n: j for j, n in enumerate(DD)}

_TABLES_PATCHED = False


def _patch_act_tables():
    """Strip the functions of natural_log_exp_and_others from every other
    activation table set before the act-table-load CFG pass runs, so all
    activations resolve to that single set and the scalar engine loads its
    tables exactly once."""
    global _TABLES_PATCHED
    if _TABLES_PATCHED:
        return
    import concourse.bacc as bacc
    from concourse import hw_specs

    _orig = hw_specs.get_activation_tables
    target = "natural_log_exp_and_others"

    def _combined_only(arch):
        tables = _orig(arch)
        if target in tables:
            keep = tables[target]
            for name in list(tables):
                if name != target:
                    tables[name] = tables[name] - keep
        return tables

    bacc.get_activation_tables = _combined_only
    _TABLES_PATCHED = True


def build_program(T=T_FULL, tc_len=TC):
    _patch_act_tables()
    import concourse.bass as bass
    import concourse.bacc as bacc
    import concourse.mybir as mybir
    import concourse.tile as tile

    F32 = mybir.dt.float32
    op = mybir.AluOpType
    AF = mybir.ActivationFunctionType

    nc = bacc.Bacc("TRN2")
    dd = nc.declare_dram_parameter("dd", [len(DD), PPART, T, CL * M], F32,
                                   isOutput=False)
    pet = nc.declare_dram_parameter("pet", [PPART, T, CL], F32, isOutput=False)
    uh = nc.declare_dram_parameter("uh", [PPART, LENF * CL], F32, isOutput=False)
    qr = nc.declare_dram_parameter("qr", [PPART, T, CL], F32, isOutput=True)

    chunks = [(t0, min(tc_len, T - t0)) for t0 in range(0, T, tc_len)]

    with tile.TileContext(nc) as tctx:
        with (
            tctx.tile_pool(name="blk", bufs=3) as blk_pool,
            tctx.tile_pool(name="st", bufs=6) as st_pool,
            tctx.tile_pool(name="per", bufs=1) as per_pool,
        ):
            V = nc.vector
            G = nc.gpsimd
            A = nc.scalar
            S = nc.sync

            def tt(eng, out, a, b, o):
                eng.tensor_tensor(out, a, b, o)

            Qfull = per_pool.tile([PPART, (LENF - 1 + T) * CL], F32)
            uh_t = per_pool.tile([PPART, LENF * CL], F32)
            S.dma_start(uh_t[:], uh[:])
            G.memset(Qfull[:, : (LENF - 1) * CL], 0.0)

            state = {}
            for s in ("SP", "SM"):
                t_ = st_pool.tile([PPART, 16], F32, tag=s)
                G.memset(t_[:], 0.001)
                state[s] = t_
            t_ = st_pool.tile([PPART, 16], F32, tag="NMW")
            G.memset(t_[:], -0.001)
            state["NMW"] = t_
            # comb holds [SUZ2 | SLZ2 | -SUZ' | -SLZ'] per step; the last two
            # 16-lane blocks are the (negated) response states.
            comb0 = st_pool.tile([PPART, 64], F32, tag="comb")
            G.memset(comb0[:, 32:64], -0.001)
            state["NSUZ"] = comb0[:, 32:48]
            state["NSLZ"] = comb0[:, 48:64]

            def nt(tag):
                return st_pool.tile([PPART, 16], F32, tag=tag, name=tag)

            def emit_dma(ci):
                t0, tcn = chunks[ci]
                n16 = tcn * 16
                ck = {"t0": t0, "tcn": tcn}
                for name in DD:
                    dt_ = blk_pool.tile([PPART, tc_len * 16], F32, tag=name,
                                        name=f"{name}_{t0}")
                    S.dma_start(
                        dt_[:, :n16].rearrange("p (t f) -> p t f", f=16),
                        dd[DJ[name], :, t0 : t0 + tcn, :],
                    )
                    ck[name] = dt_
                pt = blk_pool.tile([PPART, tc_len * CL], F32, tag="PET",
                                   name=f"PET_{t0}")
                S.dma_start(
                    pt[:, : tcn * CL].rearrange("p (t c) -> p t c", c=CL),
                    pet[:, t0 : t0 + tcn, :],
                )
                ck["PET"] = pt
                ck["PETb"] = (
                    pt[:, : tcn * CL]
                    .rearrange("p (t c) -> p t c", c=CL)
                    .unsqueeze(3)
                    .to_broadcast((PPART, tcn, CL, M))
                )
                return ck

            cur = emit_dma(0)
            pendQ = None
            pendR = None

            def emit_pendR(p):
                """Deferred q-dependent response tail of the previous step,
                emitted after the next step's snow block so the Pool engine
                never stalls mid-step waiting for the DVE's q."""
                if p is None:
                    return
                comb_p = p["comb"]
                Q0 = nt("Q0")
                tt(G, Q0[:], p["K0"], p["q"][:], op.mult)
                SUZ3 = nt("SUZ3")
                tt(G, SUZ3[:], comb_p[:, 0:16], Q0[:], op.subtract)
                NSUZn = comb_p[:, 32:48]
                tt(G, NSUZn, p["K1Cn"], SUZ3[:], op.mult)  # (K1-1)*SUZ3
                state["NSUZ"] = NSUZn
                NSLZn = comb_p[:, 48:64]
                tt(G, NSLZn, p["K2Cn"], comb_p[:, 16:32], op.mult)
                state["NSLZ"] = NSLZn

            def emit_pendQ(p):
                if p is None:
                    return
                # Q0+Q1+Q2 per cell = sum over {group, m} of
                # [SUZ2 | SLZ2 | -SUZ' | -SLZ'] — one strided-view reduce.
                V.tensor_reduce(
                    Qfull[:, (LENF - 1 + p["t"]) * CL : (LENF + p["t"]) * CL],
                    p["comb"][:].rearrange("p (g c m) -> p c g m", g=4, m=M),
                    axis=mybir.AxisListType.XY,
                    op=op.add,
                )

            for ci in range(len(chunks)):
                nxt = emit_dma(ci + 1) if ci + 1 < len(chunks) else None
                t0, tcn = cur["t0"], cur["tcn"]

                for ti in range(tcn):
                    t = t0 + ti
                    sl = slice(ti * 16, (ti + 1) * 16)

                    def cs(name):
                        return cur[name][:, sl]

                    SP, NMW = state["SP"], state["NMW"]
                    SM = state["SM"]

                    # -- kick off the soil ACT chain for this step --
                    lnSM = nt("lnSM")
                    A.activation(lnSM[:], SM[:], AF.Ln)

                    # -- snow (fills the lnSM window) --
                    SPa = nt("SPa")
                    tt(G, SPa[:], SP[:], cs("SNOW"), op.add)
                    mn = nt("mn")
                    tt(V, mn[:], cs("E"), SPa[:], op.min)
                    X = nt("X")
                    tt(V, X[:], mn[:], NMW[:], op.max)
                    SPn = nt("SP")
                    tt(G, SPn[:], SPa[:], X[:], op.subtract)
                    state["SP"] = SPn
                    NMW2 = nt("NMW2")
                    tt(G, NMW2[:], NMW[:], X[:], op.subtract)
                    NW = nt("NW")
                    tt(G, NW[:], cs("CWHn"), SPn[:], op.mult)  # = -CWH*SP
                    dw = nt("dw")
                    tt(G, dw[:], NW[:], NMW2[:], op.subtract)
                    tosp = nt("tosp")
                    V.tensor_scalar_max(tosp[:], dw[:], 0.0)
                    NMWn = nt("NMW")
                    tt(V, NMWn[:], NMW2[:], NW[:], op.max)
                    state["NMW"] = NMWn
                    wi = nt("wi")
                    tt(G, wi[:], cs("RAIN"), tosp[:], op.add)
                    SMa = nt("SMa")
                    tt(G, SMa[:], SM[:], wi[:], op.add)

                    # previous step's deferred response tail, then its
                    # consumers for this step
                    emit_pendR(pendR)
                    NSUZ, NSLZ = state["NSUZ"], state["NSLZ"]
                    CnSLZ = nt("CnSLZ")
                    tt(G, CnSLZ[:], cs("C"), NSLZ, op.mult)  # = -C*SLZ
                    SUZ1a = nt("SUZ1a")
                    tt(G, SUZ1a[:], SMa[:], NSUZ, op.subtract)

                    # -- on-path: u = BETA*lnSM - BLF --
                    v = nt("v")
                    tt(V, v[:], lnSM[:], cs("BETA"), op.mult)
                    u = nt("u")
                    tt(V, u[:], v[:], cs("BLF"), op.subtract)
                    x1 = nt("x1")
                    A.activation(x1[:], u[:], AF.Exp)

                    # x1 window: previous step's Q output
                    emit_pendQ(pendQ)

                    # -- on-path: recharge, SM1 --
                    rech = nt("rech")
                    V.scalar_tensor_tensor(rech[:], x1[:], 1.0, wi[:],
                                           op.min, op.mult)
                    SM1 = nt("SM1")
                    tt(V, SM1[:], SMa[:], rech[:], op.subtract)
                    ln2 = nt("ln2")
                    A.activation(ln2[:], SM1[:], AF.Ln)

                    # ln2 window: SMc and the response head
                    SMc = nt("SMc")
                    tt(V, SMc[:], SM1[:], cs("FC"), op.min)
                    SMcP = nt("SMcP")
                    tt(G, SMcP[:].rearrange("p (c m) -> p c m", m=M),
                       SMc[:].rearrange("p (c m) -> p c m", m=M),
                       cur["PETb"][:, ti, :, :], op.subtract)
                    SMcP2 = nt("SMcP2")
                    V.tensor_scalar_max(SMcP2[:], SMcP[:], NZ)
                    SUZ1 = nt("SUZ1")
                    tt(G, SUZ1[:], SUZ1a[:], SMc[:], op.subtract)
                    PERCa = nt("PERCa")
                    tt(V, PERCa[:], SUZ1[:], cs("PERC"), op.min)
                    comb = st_pool.tile([PPART, 64], F32, tag="comb",
                                        name="comb")
                    SUZ2 = comb[:, 0:16]
                    tt(G, SUZ2, SUZ1[:], PERCa[:], op.subtract)
                    t5 = nt("t5")
                    tt(G, t5[:], SUZ2, cs("NUZL"), op.add)
                    q = nt("q")
                    V.tensor_scalar_max(q[:], t5[:], 0.0)

                    # -- on-path: w2 = BETAET*ln2 + LNPB --
                    v2 = nt("v2")
                    tt(V, v2[:], ln2[:], cs("BETAET"), op.mult)
                    w2 = nt("w2")
                    tt(V, w2[:], v2[:], cs("LNPB"), op.add)
                    x2 = nt("x2")
                    A.activation(x2[:], w2[:], AF.Exp)

                    # -- on-path tail: SM3, capillary, SM --
                    tq = nt("tq")
                    V.scalar_tensor_tensor(tq[:], x2[:], -1.0, SMc[:],
                                           op.mult, op.add)
                    SM3 = nt("SM3")
                    tt(V, SM3[:], tq[:], SMcP2[:], op.max)
                    g = nt("g")
                    tt(V, g[:], SM3[:], cs("FCinv"), op.mult)
                    rln = nt("rln")
                    V.tensor_scalar(rln[:], g[:], 1.0, 1.0, op.min, op.subtract)
                    cap = nt("cap")
                    tt(V, cap[:], CnSLZ[:], rln[:], op.mult)
                    SMn = nt("SM")
                    tt(V, SMn[:], SM3[:], cap[:], op.add)
                    state["SM"] = SMn

                    # -- response tail --
                    sl_n = nt("sl_n")
                    tt(V, sl_n[:], NSLZ, cap[:], op.add)
                    NSLZ1 = nt("NSLZ1")
                    V.tensor_scalar_min(NSLZ1[:], sl_n[:], -NZ)
                    SLZ2 = comb[:, 16:32]
                    tt(V, SLZ2, PERCa[:], NSLZ1[:], op.subtract)

                    pendR = {"comb": comb, "q": q, "K0": cs("K0"),
                             "K1Cn": cs("K1Cn"), "K2Cn": cs("K2Cn")}
                    pendQ = {"t": t, "comb": comb}

                if nxt is not None:
                    cur = nxt

            emit_pendR(pendR)
            emit_pendQ(pendQ)

            # ---- gamma-UH routing (DVE, bulk) ----
            Qr = per_pool.tile([PPART, T * CL], F32)
            prod = per_pool.tile([PPART, T * CL], F32)

            def qr4(ap_):
                return ap_.rearrange("p (t c) -> p t c", c=CL)

            for k in range(LENF):
                sh = Qfull[:, (LENF - 1 - k) * CL : (LENF - 1 - k + T) * CL]
                uhk = (
                    uh_t[:, k * CL : (k + 1) * CL]
                    .unsqueeze(1)
                    .to_broadcast((PPART, T, CL))
                )
                if k == 0:
                    tt(V, qr4(Qr[:]), uhk, qr4(sh), op.mult)
                else:
                    tt(V, qr4(prod[:]), uhk, qr4(sh), op.mult)
                    tt(V, qr4(Qr[:]), qr4(Qr[:]), qr4(prod[:]), op.add)

            S.dma_start(qr[:, :, :], Qr[:].rearrange("p (t c) -> p t c", c=CL))

    return nc


# ---------------- host-side packing ----------------

def _derived_full(x_hydro_model, params_raw):
    """All state-free per-step tensors, float32, shapes [T, N, M] (per-cell
    quantities broadcast over M)."""
    f32 = np.float32
    T, N, _ = x_hydro_model.shape
    raw = np.ascontiguousarray(params_raw[:, :, :14, :], dtype=f32)
    x = np.ascontiguousarray(x_hydro_model, dtype=f32)
    P = x[:, :, 0:1]
    Ta = x[:, :, 1:2]
    PET = x[:, :, 2:3]

    BETA = f32(5.0) * raw[:, :, 0] + f32(1.0)
    FC = f32(950.0) * raw[:, :, 1] + f32(50.0)
    K0 = f32(0.85) * raw[:, :, 2] + f32(0.05)
    K1Cn = f32(0.49) * raw[:, :, 3] - f32(0.99)
    K2Cn = f32(0.199) * raw[:, :, 4] - f32(0.999)
    LP = f32(0.8) * raw[:, :, 5] + f32(0.2)
    PERC = f32(10.0) * raw[:, :, 6]
    NUZL = f32(-100.0) * raw[:, :, 7]
    TTn = f32(-5.0) * raw[:, :, 8] + f32(2.5)
    CFMX = f32(9.5) * raw[:, :, 9] + f32(0.5)
    CWHn = f32(-0.2) * raw[:, :, 11]
    BETAET = f32(4.7) * raw[:, :, 12] + f32(0.3)
    C = raw[:, :, 13]

    Tdiff = (Ta + TTn).astype(f32)
    m1 = (CFMX * Tdiff).astype(f32)
    rn = np.maximum(-m1, 0).astype(f32)
    Rc0 = ((f32(0.1) * raw[:, :, 10]).astype(f32) * rn).astype(f32)
    Gc0 = np.maximum(m1, 0).astype(f32)
    E = (Gc0 - Rc0).astype(f32)
    mask = (Tdiff >= 0).astype(f32)
    RAIN = (mask * P).astype(f32)
    SNOW = (P - RAIN).astype(f32)
    lnFC = np.log(FC).astype(f32)
    FCinv = np.exp(-lnFC).astype(f32)
    BLF = (BETA * lnFC).astype(f32)
    LPFC = (LP * FC).astype(f32)
    lnLPFC = np.log(LPFC).astype(f32)
    BL2 = (BETAET * lnLPFC).astype(f32)
    lnPET = np.log(np.maximum(PET, f32(1e-30))).astype(f32)
    LNPB = (lnPET - BL2).astype(f32)

    return {
        "E": E, "SNOW": SNOW, "RAIN": RAIN, "CWHn": CWHn, "BETA": BETA,
        "BLF": BLF, "FC": FC, "FCinv": FCinv, "BETAET": BETAET, "LNPB": LNPB,
        "C": C, "PERC": PERC, "NUZL": NUZL, "K0": K0, "K1Cn": K1Cn,
        "K2Cn": K2Cn,
    }


def pack_inputs(x_hydro_model, params_raw, conv_params_hydro):
    T = x_hydro_model.shape[0]
    f32 = np.float32
    der = _derived_full(x_hydro_model, params_raw)
    # [T, N, M] -> per core [PPART, T, CL*M]
    dd_full = np.stack([der[n] for n in DD], axis=0)  # [nd, T, N, M]
    nd = dd_full.shape[0]
    dd_c = dd_full.reshape(nd, T, NCORES, PPART, CL * M).transpose(2, 0, 3, 1, 4)

    PET = np.ascontiguousarray(x_hydro_model[:, :, 2], dtype=f32)  # [T, N]
    pet_c = PET.reshape(T, NCORES, PPART, CL).transpose(1, 2, 0, 3)

    conv = np.asarray(conv_params_hydro, dtype=np.float64)
    a = conv[:, 0] * 2.9
    b = conv[:, 1] * 6.5
    aa = np.maximum(a, 0) + 0.1
    theta = np.maximum(b, 0) + 0.5
    tgrid = np.arange(0.5, float(LENF), dtype=np.float64)[:, None]
    lg = np.array([math.lgamma(v) for v in aa])
    w = np.exp(-lg) / theta ** aa * tgrid ** (aa - 1.0) * np.exp(-tgrid / theta)
    w = w / w.sum(0)
    UH = (w * (1.0 / M)).astype(f32)  # [LENF, NGRID], mean-over-M folded in
    uh_c = UH.reshape(LENF, NCORES, PPART, CL).transpose(1, 2, 0, 3)

    in_maps = []
    for i in range(NCORES):
        in_maps.append({
            "dd": np.ascontiguousarray(dd_c[i]),
            "pet": np.ascontiguousarray(pet_c[i]),
            "uh": np.ascontiguousarray(uh_c[i]).reshape(PPART, LENF * CL),
        })
    return in_maps


def unpack_outputs(results, T):
    out = np.empty((T, NGRID), np.float32)
    for i in range(NCORES):
        q = results[i]["qr"].reshape(PPART, T, CL)
        out[:, i * NSH : (i + 1) * NSH] = q.transpose(1, 0, 2).reshape(T, NSH)
    return out


_PROG_CACHE = {}


def kernel(x_hydro_model, params_raw, conv_params_hydro):
    from concourse.bass_utils import run_bass_kernel_spmd

    T = x_hydro_model.shape[0]
    key = T
    if key not in _PROG_CACHE:
        _PROG_CACHE[key] = build_program(T=T)
    nc = _PROG_CACHE[key]
    if not nc.is_finalized():
        nc.finalize()
    in_maps = pack_inputs(x_hydro_model, params_raw, conv_params_hydro)
    res = run_bass_kernel_spmd(nc, in_maps, list(range(NCORES)))
    return unpack_outputs(res.results, T)
